# revision 6
# baseline (speedup 1.0000x reference)
"""Trainium2 kernel for nn_HDNet_52707838657074.

Reference computation:
    m = x0 @ W
    U, S, Vh = svd(m, full_matrices=False)     (CPU LAPACK gesdd)
    P, L, Uu = lu(x1)                          (CPU LAPACK getrf)
    a = (U + L) / 2 ; b = max(Vh, Uu)
    out = (a @ b) * S + P @ a

The SVD/LU factors are only defined up to LAPACK's internal sign/pivot
choices, and the tail mixes factors non-invariantly (U with L, Vh with Uu),
so the factorization must reproduce the reference's LAPACK path bit-exactly:
it runs on the host CPU jax backend. The remaining O(n^3) work -- the
[4096,1024]x[1024,1024] matmul, the S column scaling and the P@a
permutation-add -- runs on the 8 NeuronCores, data-parallel over rows
(512 rows/core, b*S replicated), as:  out = a @ (b*S) + a[pidx].

Device-side the matmul runs in split precision: a and bS/32 are split on
the host into fp16 hi+lo pairs (a = ah + al + O(2^-22)), and the PE computes
ah@bh + ah@bl + al@bh at 1 cycle/row (vs 4 for native fp32) accumulating in
fp32 PSUM. fp16 products are exact in fp32, so the only extra error is the
dropped al@bl term (~2^-22 relative) -- measured end-to-end accuracy equals
the native-f32 kernel (~6e-7 scale-relative) at ~2.3x the speed. The hi+lo
fp16 pair is 4 bytes/element, so DMA bytes equal the f32 kernel's.
The 1/32 prescale (undone in the PSUM eviction) keeps max|bS| = ~68e3
under fp16's 65504 max.
"""

import numpy as np

B, C = 4096, 1024
NCORES = 8
MB = B // NCORES          # 512 rows per core
P = 128                   # partitions
KT = C // P               # 8 k-tiles
MT = MB // P              # 4 m-tiles
S_SCALE = 32.0

_STATE = {}


def _patch_tile_teardown():
    """Drop TileContext's exit-time semaphore RANGE_CLEAR + second barrier.

    The program epilogue (emitted after the Tile block) already zeroes every
    semaphore on all engines, so Tile's own clear is redundant; keeping one
    barrier after the drain preserves the all-engines-quiesced invariant.
    """
    import concourse.tile as tile_mod

    if getattr(tile_mod.TileContext, "_fast_teardown", False):
        return
    ScopedClock = tile_mod.ScopedClock

    def _drain_and_barrier(self, tick_clock, wait_clock):
        drain_inst = self.nc.sync.drain()
        wait_clock.add_sem_waits(
            drain_inst.ins, ScopedClock({None: tick_clock.global_clock})
        )
        self.nc.all_engine_barrier()
        popped = self.nc._tile_sem_poison_stack.pop()
        assert popped is self._sem_poison
        for h in self.sems.allocated().values():
            self.nc.release_semaphore(h)

    tile_mod.TileContext._drain_and_barrier = _drain_and_barrier
    tile_mod.TileContext._fast_teardown = True


def _build_nc():
    import concourse.mybir as mybir
    import concourse.tile as tile
    from concourse import bacc
    from concourse.bass import ds, ts
    from concourse.kernels.tile_matmul import (
        ShapeInfo,
        composable_matmul_tile_kernel,
    )

    _patch_tile_teardown()

    f16 = mybir.dt.float16
    f32 = mybir.dt.float32

    nc = bacc.Bacc(None, target_bir_lowering=False, debug=False)
    # Layouts follow tile_matmul convention: [K,M] matrix fed as [P, K//P, M]
    # with matrix row k = kt*P + p stored at [p, kt, :].
    ahm = nc.declare_dram_parameter("ahm", [P, KT, MB], f16, isOutput=False)
    alm = nc.declare_dram_parameter("alm", [P, KT, MB], f16, isOutput=False)
    bhn = nc.declare_dram_parameter("bhn", [P, KT, C], f16, isOutput=False)
    bln = nc.declare_dram_parameter("bln", [P, KT, C], f16, isOutput=False)
    acc = nc.declare_dram_parameter("acc", [P, MT, C], f32, isOutput=False)
    out = nc.declare_dram_parameter("out", [P, MT, C], f32, isOutput=True)

    with tile.TileContext(nc) as tc:
        import contextlib

        with contextlib.ExitStack() as ctx:
            kxm_pool = ctx.enter_context(tc.tile_pool(name="kxm_pool", bufs=KT + 1))
            kxn_pool = ctx.enter_context(tc.tile_pool(name="kxn_pool", bufs=2 * KT + 1))
            acc_pool = ctx.enter_context(tc.tile_pool(name="acc_pool", bufs=8))

            # Three K-batches: (ah,bh), (ah,bl), (al,bh). Producers cache
            # SBUF tiles so each hi/lo tensor is DMA'd exactly once.
            kxm_cache = {}
            kxn_cache = {}

            def kxm_producer(nc_, md):
                which = 0 if md.k_batch_idx < 2 else 1       # hi, hi, lo
                key = (which, md.k_tile_idx)
                t = kxm_cache.get(key)
                if t is None:
                    t = kxm_pool.tile([P, md.k_subtiles, md.m_tile], f16,
                                      name=f"kxm{which}")
                    src = ahm if which == 0 else alm
                    # scalar-engine DGE queue: keeps descriptor pushes off
                    # Sync, which handles kxn + stores.
                    nc_.scalar.dma_start(
                        out=t,
                        in_=src[:, md.k_tile_idx:md.k_tile_idx + md.k_subtiles,
                                ts(md.m_tile_idx, md.m_tile)])
                    kxm_cache[key] = t
                return t

            def kxn_producer(nc_, md):
                which = 0 if md.k_batch_idx != 1 else 1      # hi, lo, hi
                key = (which, md.k_tile_idx, md.n_tile_idx)
                t = kxn_cache.get(key)
                if t is None:
                    t = kxn_pool.tile([P, md.k_subtiles, md.n_tile], f16,
                                      name=f"kxn{which}")
                    src = bhn if which == 0 else bln
                    eng = nc_.sync if (md.k_tile_idx % 2 == 0) else nc_.gpsimd
                    eng.dma_start(
                        out=t,
                        in_=src[:, md.k_tile_idx:md.k_tile_idx + md.k_subtiles,
                                ts(md.n_tile_idx, md.n_tile)])
                    kxn_cache[key] = t
                return t

            # PSUM eviction fused with the *S_SCALE rescale and +acc add,
            # then immediate per-chunk store.
            def reducer(nc_, psum, sbuf, md):
                mt = md.m_tile_idx * md.m_subtiles + md.m_subtile_idx
                col = md.n_tile_idx * md.n_tile + md.n_subtile_idx * md.n_subtile
                w = md.n_slice_size
                acc_tile = acc_pool.tile([P, 1, md.n_subtile], f32, name="acc")
                nc_.scalar.dma_start(out=acc_tile[:, :, :w],
                                     in_=acc[:, mt:mt + 1, ds(col, w)])
                nc_.vector.scalar_tensor_tensor(
                    out=sbuf, in0=psum, scalar=S_SCALE,
                    in1=acc_tile[:, :, :w],
                    op0=mybir.AluOpType.mult, op1=mybir.AluOpType.add)
                nc_.sync.dma_start(out=out[:, mt:mt + 1, ds(col, w)], in_=sbuf)

            composable_matmul_tile_kernel(
                tc=tc,
                kxm_shape=ShapeInfo(pdims=[(P, KT)] * 3, fdims=[MB]),
                kxn_shape=ShapeInfo(pdims=[(P, KT)] * 3, fdims=[C]),
                output_type=f32,
                kxm_producer=kxm_producer,
                kxn_producer=kxn_producer,
                mxn_subtile_reducer=reducer,
                mxn_consumer=lambda nc_, sbuf, md: None,
                MATMUL_FREE_DIM=512,
                MAX_TILE_SIZE=512,
                MAX_K_TILE_SIZE=128,
                cache_tiles=True,
                psum_n_bufs=2,
            )
    nc.compile()
    return nc


def _split_f16(x):
    hi = x.astype(np.float16)
    lo = (x - hi.astype(np.float32)).astype(np.float16)
    return hi, lo


def _kxm_layout(x):
    # [MB, C] -> [P, KT, MB] with element [p, kt, m] = x[m, kt*P + p]
    return np.ascontiguousarray(x.reshape(MB, KT, P).transpose(2, 1, 0))


def _kxn_layout(x):
    # [C, C] -> [P, KT, C] with element [p, kt, n] = x[kt*P + p, n]
    return np.ascontiguousarray(x.reshape(KT, P, C).transpose(1, 0, 2))


def _mxn_layout(x):
    # [MB, C] -> [P, MT, C] with element [p, mt, n] = x[mt*P + p, n]
    return np.ascontiguousarray(x.reshape(MT, P, C).transpose(1, 0, 2))


def _device_matmul_add(a, bS, ap, trace=False, **kwargs):
    """out[MB*8, C] = a @ bS + ap on 8 NeuronCores, row-sharded."""
    from concourse import bass_utils

    if "nc" not in _STATE:
        _STATE["nc"] = _build_nc()
    nc = _STATE["nc"]

    bh, bl = _split_f16(bS * np.float32(1.0 / S_SCALE))
    bhn_g = _kxn_layout(bh)
    bln_g = _kxn_layout(bl)
    in_maps = []
    for c in range(NCORES):
        a_sh = a[c * MB:(c + 1) * MB]
        ah, al = _split_f16(a_sh)
        in_maps.append({
            "ahm": _kxm_layout(ah),
            "alm": _kxm_layout(al),
            "bhn": bhn_g,
            "bln": bln_g,
            "acc": _mxn_layout(ap[c * MB:(c + 1) * MB]),
        })

    res = bass_utils.run_bass_kernel_spmd(
        nc, in_maps, core_ids=list(range(NCORES)), trace=trace, **kwargs
    )
    outs = []
    for c in range(NCORES):
        o = res.results[c]["out"]  # [P, MT, C]
        outs.append(np.asarray(o).transpose(1, 0, 2).reshape(MB, C))
    return np.concatenate(outs, axis=0), res


def _host_factorizations(x0, x1, W):
    import jax
    import jax.numpy as jnp
    from jax.scipy.linalg import lu as jax_lu

    cpu = jax.local_devices(backend="cpu")[0]
    with jax.default_device(cpu):
        x0j = jnp.asarray(np.asarray(x0))
        x1j = jnp.asarray(np.asarray(x1))
        Wj = jnp.asarray(np.asarray(W))
        m = x0j @ Wj
        U, S, Vh = jnp.linalg.svd(m, full_matrices=False)
        Pm, L, Uu = jax_lu(x1j)
        a = (U + L) * 0.5
        b = jnp.maximum(Vh, Uu)
        a_np = np.asarray(a)
        b_np = np.asarray(b)
        S_np = np.asarray(S)
        P_np = np.asarray(Pm)
    pidx = np.argmax(P_np, axis=1)          # (P@a)[i] == a[pidx[i]], exact
    return a_np, b_np * S_np[None, :], a_np[pidx]


def kernel(x0, x1, W):
    a, bS, ap = _host_factorizations(x0, x1, W)
    out, _ = _device_matmul_add(a, bS, ap)
    return np.ascontiguousarray(out, dtype=np.float32)


# revision 7
# speedup vs baseline: 1.0580x; 1.0580x over previous
"""Trainium2 kernel for nn_HDNet_52707838657074.

Reference computation:
    m = x0 @ W
    U, S, Vh = svd(m, full_matrices=False)     (CPU LAPACK gesdd)
    P, L, Uu = lu(x1)                          (CPU LAPACK getrf)
    a = (U + L) / 2 ; b = max(Vh, Uu)
    out = (a @ b) * S + P @ a

The SVD/LU factors are only defined up to LAPACK's internal sign/pivot
choices, and the tail mixes factors non-invariantly (U with L, Vh with Uu),
so the factorization must reproduce the reference's LAPACK path bit-exactly:
it runs on the host CPU jax backend. The remaining O(n^3) work -- the
[4096,1024]x[1024,1024] matmul, the S column scaling and the P@a
permutation-add -- runs on the 8 NeuronCores, data-parallel over rows
(512 rows/core, b*S replicated), as:  out = a @ (b*S) + a[pidx].

Device-side the matmul runs in split precision: a and bS/32 are split on
the host into fp16 hi+lo pairs (a = ah + al + O(2^-22)), and the PE computes
ah@bh + ah@bl + al@bh at 1 cycle/row (vs 4 for native fp32) accumulating in
fp32 PSUM. fp16 products are exact in fp32, so the only extra error is the
dropped al@bl term (~2^-22 relative) -- measured end-to-end accuracy equals
the native-f32 kernel (~6e-7 scale-relative) at ~2.3x the speed. The hi+lo
fp16 pair is 4 bytes/element, so DMA bytes equal the f32 kernel's.
The 1/32 prescale (undone in the PSUM eviction) keeps max|bS| = ~68e3
under fp16's 65504 max.
"""

import numpy as np

B, C = 4096, 1024
NCORES = 8
MB = B // NCORES          # 512 rows per core
P = 128                   # partitions
KT = C // P               # 8 k-tiles
MT = MB // P              # 4 m-tiles
S_SCALE = 32.0

_STATE = {}


def _patch_tile_teardown():
    """Drop TileContext's exit-time semaphore RANGE_CLEAR + second barrier.

    The program epilogue (emitted after the Tile block) already zeroes every
    semaphore on all engines, so Tile's own clear is redundant; keeping one
    barrier after the drain preserves the all-engines-quiesced invariant.
    """
    import concourse.tile as tile_mod

    if getattr(tile_mod.TileContext, "_fast_teardown", False):
        return
    ScopedClock = tile_mod.ScopedClock

    def _drain_and_barrier(self, tick_clock, wait_clock):
        drain_inst = self.nc.sync.drain()
        wait_clock.add_sem_waits(
            drain_inst.ins, ScopedClock({None: tick_clock.global_clock})
        )
        self.nc.all_engine_barrier()
        popped = self.nc._tile_sem_poison_stack.pop()
        assert popped is self._sem_poison
        for h in self.sems.allocated().values():
            self.nc.release_semaphore(h)

    tile_mod.TileContext._drain_and_barrier = _drain_and_barrier
    tile_mod.TileContext._fast_teardown = True


def _build_nc():
    import concourse.mybir as mybir
    import concourse.tile as tile
    from concourse import bacc
    from concourse.bass import ds, ts
    from concourse.kernels.tile_matmul import (
        ShapeInfo,
        composable_matmul_tile_kernel,
    )

    _patch_tile_teardown()

    f16 = mybir.dt.float16
    f32 = mybir.dt.float32

    nc = bacc.Bacc(None, target_bir_lowering=False, debug=False)
    # Layouts follow tile_matmul convention: [K,M] matrix fed as [P, K//P, M]
    # with matrix row k = kt*P + p stored at [p, kt, :].
    ahm = nc.declare_dram_parameter("ahm", [P, KT, MB], f16, isOutput=False)
    alm = nc.declare_dram_parameter("alm", [P, KT, MB], f16, isOutput=False)
    bhn = nc.declare_dram_parameter("bhn", [P, KT, C], f16, isOutput=False)
    bln = nc.declare_dram_parameter("bln", [P, KT, C], f16, isOutput=False)
    acc = nc.declare_dram_parameter("acc", [P, MT, C], f32, isOutput=False)
    out = nc.declare_dram_parameter("out", [P, MT, C], f32, isOutput=True)

    with tile.TileContext(nc) as tc:
        import contextlib

        with contextlib.ExitStack() as ctx:
            kxm_pool = ctx.enter_context(tc.tile_pool(name="kxm_pool", bufs=KT + 1))
            kxn_pool = ctx.enter_context(tc.tile_pool(name="kxn_pool", bufs=2 * KT + 1))
            acc_pool = ctx.enter_context(tc.tile_pool(name="acc_pool", bufs=8))

            # Three K-batches: (ah,bh), (ah,bl), (al,bh). Producers cache
            # SBUF tiles so each hi/lo tensor is DMA'd exactly once.
            kxm_cache = {}
            kxn_cache = {}

            def kxm_producer(nc_, md):
                which = 0 if md.k_batch_idx < 2 else 1       # hi, hi, lo
                key = (which, md.k_tile_idx)
                t = kxm_cache.get(key)
                if t is None:
                    t = kxm_pool.tile([P, md.k_subtiles, md.m_tile], f16,
                                      name=f"kxm{which}")
                    src = ahm if which == 0 else alm
                    nc_.sync.dma_start(
                        out=t,
                        in_=src[:, md.k_tile_idx:md.k_tile_idx + md.k_subtiles,
                                ts(md.m_tile_idx, md.m_tile)])
                    kxm_cache[key] = t
                return t

            def kxn_producer(nc_, md):
                which = 0 if md.k_batch_idx != 1 else 1      # hi, lo, hi
                key = (which, md.k_tile_idx, md.n_tile_idx)
                t = kxn_cache.get(key)
                if t is None:
                    t = kxn_pool.tile([P, md.k_subtiles, md.n_tile], f16,
                                      name=f"kxn{which}")
                    src = bhn if which == 0 else bln
                    nc_.sync.dma_start(
                        out=t,
                        in_=src[:, md.k_tile_idx:md.k_tile_idx + md.k_subtiles,
                                ts(md.n_tile_idx, md.n_tile)])
                    kxn_cache[key] = t
                return t

            # PSUM eviction fused with the *S_SCALE rescale and +acc add,
            # then immediate per-chunk store.
            def reducer(nc_, psum, sbuf, md):
                mt = md.m_tile_idx * md.m_subtiles + md.m_subtile_idx
                col = md.n_tile_idx * md.n_tile + md.n_subtile_idx * md.n_subtile
                w = md.n_slice_size
                acc_tile = acc_pool.tile([P, 1, md.n_subtile], f32, name="acc")
                nc_.sync.dma_start(out=acc_tile[:, :, :w],
                                   in_=acc[:, mt:mt + 1, ds(col, w)])
                nc_.vector.scalar_tensor_tensor(
                    out=sbuf, in0=psum, scalar=S_SCALE,
                    in1=acc_tile[:, :, :w],
                    op0=mybir.AluOpType.mult, op1=mybir.AluOpType.add)
                nc_.sync.dma_start(out=out[:, mt:mt + 1, ds(col, w)], in_=sbuf)

            composable_matmul_tile_kernel(
                tc=tc,
                kxm_shape=ShapeInfo(pdims=[(P, KT)] * 3, fdims=[MB]),
                kxn_shape=ShapeInfo(pdims=[(P, KT)] * 3, fdims=[C]),
                output_type=f32,
                kxm_producer=kxm_producer,
                kxn_producer=kxn_producer,
                mxn_subtile_reducer=reducer,
                mxn_consumer=lambda nc_, sbuf, md: None,
                MATMUL_FREE_DIM=512,
                MAX_TILE_SIZE=512,
                MAX_K_TILE_SIZE=128,
                cache_tiles=True,
                psum_n_bufs=2,
            )
    nc.compile()
    return nc


def _split_f16(x):
    hi = x.astype(np.float16)
    lo = (x - hi.astype(np.float32)).astype(np.float16)
    return hi, lo


def _kxm_layout(x):
    # [MB, C] -> [P, KT, MB] with element [p, kt, m] = x[m, kt*P + p]
    return np.ascontiguousarray(x.reshape(MB, KT, P).transpose(2, 1, 0))


def _kxn_layout(x):
    # [C, C] -> [P, KT, C] with element [p, kt, n] = x[kt*P + p, n]
    return np.ascontiguousarray(x.reshape(KT, P, C).transpose(1, 0, 2))


def _mxn_layout(x):
    # [MB, C] -> [P, MT, C] with element [p, mt, n] = x[mt*P + p, n]
    return np.ascontiguousarray(x.reshape(MT, P, C).transpose(1, 0, 2))


def _device_matmul_add(a, bS, ap, trace=False, **kwargs):
    """out[MB*8, C] = a @ bS + ap on 8 NeuronCores, row-sharded."""
    from concourse import bass_utils

    if "nc" not in _STATE:
        _STATE["nc"] = _build_nc()
    nc = _STATE["nc"]

    bh, bl = _split_f16(bS * np.float32(1.0 / S_SCALE))
    bhn_g = _kxn_layout(bh)
    bln_g = _kxn_layout(bl)
    in_maps = []
    for c in range(NCORES):
        a_sh = a[c * MB:(c + 1) * MB]
        ah, al = _split_f16(a_sh)
        in_maps.append({
            "ahm": _kxm_layout(ah),
            "alm": _kxm_layout(al),
            "bhn": bhn_g,
            "bln": bln_g,
            "acc": _mxn_layout(ap[c * MB:(c + 1) * MB]),
        })

    res = bass_utils.run_bass_kernel_spmd(
        nc, in_maps, core_ids=list(range(NCORES)), trace=trace, **kwargs
    )
    outs = []
    for c in range(NCORES):
        o = res.results[c]["out"]  # [P, MT, C]
        outs.append(np.asarray(o).transpose(1, 0, 2).reshape(MB, C))
    return np.concatenate(outs, axis=0), res


def _host_factorizations(x0, x1, W):
    import jax
    import jax.numpy as jnp
    from jax.scipy.linalg import lu as jax_lu

    cpu = jax.local_devices(backend="cpu")[0]
    with jax.default_device(cpu):
        x0j = jnp.asarray(np.asarray(x0))
        x1j = jnp.asarray(np.asarray(x1))
        Wj = jnp.asarray(np.asarray(W))
        m = x0j @ Wj
        U, S, Vh = jnp.linalg.svd(m, full_matrices=False)
        Pm, L, Uu = jax_lu(x1j)
        a = (U + L) * 0.5
        b = jnp.maximum(Vh, Uu)
        a_np = np.asarray(a)
        b_np = np.asarray(b)
        S_np = np.asarray(S)
        P_np = np.asarray(Pm)
    pidx = np.argmax(P_np, axis=1)          # (P@a)[i] == a[pidx[i]], exact
    return a_np, b_np * S_np[None, :], a_np[pidx]


def kernel(x0, x1, W):
    a, bS, ap = _host_factorizations(x0, x1, W)
    out, _ = _device_matmul_add(a, bS, ap)
    return np.ascontiguousarray(out, dtype=np.float32)
